# revision 23
# baseline (speedup 1.0000x reference)
"""Trainium2 Bass kernel for nn_EquivariantLinear.

Reference computation (B=65536, IN_MULT=OUT_MULT=128, DIM=9, NREPS=3):
    w3 = weight.reshape(3, 128, 128)
    wd = w3[indices]                         # (9, 128, 128)
    out = einsum('dnm,bmd->bnd', wd, f)      # (B, 128, 9)
    out[..., scalar_locs] += bias            # bias on degree-0 column(s)

Strategy (data-parallel over batch, 8 NeuronCores):
  - Each core gets B/8 = 8192 batch rows; weight/bias replicated.
  - f is (b, m, d) with d innermost, so loading "m on partitions" directly
    would make 36-byte DMA runs.  Instead DMA contiguous 128-batch-row
    tiles [128b x 1152(m,d)], transpose each per-d [128b x 128m] slice on
    the TensorEngine (PSUM), copy back to SBUF, then matmul with the
    (transposed) per-irrep weight as the moving operand:
        O_d[b, n] = T_d.T @ wT_d,  T_d = F_d.T (stationary, [m, b])
    which lands the output with batch on partitions, so the store is a
    contiguous 128-row DMA as well.
  - Bias is fused into the PSUM->SBUF copy-out (tensor_add with a
    partition-replicated bias row) for the scalar (degree-0) columns.
"""

import numpy as np

import concourse.bass as bass
import concourse.tile as tile
from concourse import bacc, mybir
from concourse.bass_utils import run_bass_kernel_spmd

FP32 = mybir.dt.float32

N_CORES = 8
B_TOTAL = 65536
B_CORE = B_TOTAL // N_CORES
M = 128  # in_mult
N = 128  # out_mult
D = 9    # sum(2l+1)
NREPS = 3
P = 128  # partitions / batch tile


def tile_schedule(ntiles128):
    """Rows-per-partition (R) per supertile: small tiles at the edges so the
    pipeline fills/drains fast, R=4 descriptors (18.4 KB) in steady state."""
    if ntiles128 < 12 or ntiles128 % 4 != 0:
        return [1] * ntiles128
    sched = [1, 1, 2]
    tail = [2, 1, 1]
    rem = ntiles128 - sum(sched) - sum(tail)
    return sched + [4] * (rem // 4) + tail


def build_nc(b_core, idx, scalar_set):
    """Build the single-core Bass program (run SPMD on all cores)."""
    nc = bacc.Bacc(None, target_bir_lowering=False, debug=True)

    f = nc.dram_tensor("f", [b_core, M, D], FP32, kind="ExternalInput")
    wdt = nc.dram_tensor("wdt", [M, NREPS * N], FP32, kind="ExternalInput")
    brow = nc.dram_tensor("brow", [P, N], FP32, kind="ExternalInput")
    ident = nc.dram_tensor("ident", [P, P], FP32, kind="ExternalInput")
    out = nc.dram_tensor("out", [b_core, N, D], FP32, kind="ExternalOutput")

    sched = tile_schedule(b_core // P)
    groups = [list(range(0, 4)), list(range(4, 8)), [8]]

    with tile.TileContext(nc) as tc:
        with (
            tc.tile_pool(name="const", bufs=1) as cpool,
            tc.tile_pool(name="fin", bufs=4) as fpool,
            tc.tile_pool(name="tsb", bufs=6) as tspool,
            tc.tile_pool(name="osb", bufs=3) as ospool,
            tc.tile_pool(name="tps", bufs=4, space=bass.MemorySpace.PSUM) as tpsum,
            tc.tile_pool(name="ops", bufs=4, space=bass.MemorySpace.PSUM) as opsum,
        ):
            # constants go over the SWDGE (gpsimd) ring so the first f-tile
            # load is not queued behind them on the sync HWDGE ring; ident
            # first (the first transposes need it before anything else)
            id_sb = cpool.tile([P, P], FP32)
            nc.gpsimd.dma_start(id_sb[:], ident[:])
            wdt_sb = cpool.tile([M, NREPS * N], FP32)
            nc.gpsimd.dma_start(wdt_sb[:], wdt[:])
            brow_sb = cpool.tile([P, N], FP32)
            nc.gpsimd.dma_start(brow_sb[:], brow[:])

            cursor = 0
            for R in sched:
                rows = P * R
                f_t = f[cursor:cursor + rows].rearrange(
                    "(p r) m d -> p (r m d)", r=R
                )
                o_t = out[cursor:cursor + rows].rearrange(
                    "(p r) n d -> p (r n d)", r=R
                )
                cursor += rows
                fb = fpool.tile([P, 4 * M * D], FP32, tag="fb")
                nc.sync.dma_start(fb[:, : R * M * D], f_t)
                fb_d = fb[:, : R * M * D].rearrange(
                    "p (r m d) -> p r d m", d=D, r=R
                )
                osb = ospool.tile([P, 4 * N * D], FP32, tag="osb")
                osb_d = osb[:, : R * N * D].rearrange(
                    "p (r n d) -> p r d n", d=D, r=R
                )

                for r in range(R):
                    for g in groups:
                        ng = len(g)
                        d0 = g[0]
                        # per-d transposes of [128b x 128m] slices into one bank
                        tps = tpsum.tile([P, 512], FP32, tag="tp")
                        for i, d in enumerate(g):
                            nc.tensor.transpose(
                                tps[:, i * P:(i + 1) * P], fb_d[:, r, d, :], id_sb[:]
                            )
                        tsb = tspool.tile([P, 512], FP32, tag="tsb")
                        nc.scalar.copy(tsb[:, : ng * P], tps[:, : ng * P])
                        # matmuls: O_d = T_d.T @ wT_{idx[d]} -> [b, n] in PSUM
                        ops = opsum.tile([P, 512], FP32, tag="op")
                        for i, d in enumerate(g):
                            nc.tensor.matmul(
                                ops[:, i * P:(i + 1) * P],
                                tsb[:, i * P:(i + 1) * P],
                                wdt_sb[:, idx[d] * N:(idx[d] + 1) * N],
                                start=True,
                                stop=True,
                            )
                        # copy out to SBUF in one op, interleaving d back
                        # into (n d) order; bias is added SBUF-in-place
                        # afterwards so every PSUM read is a uniform copy
                        ops_v = ops[:, : ng * P].rearrange("p (i n) -> p i n", i=ng)
                        nc.vector.tensor_copy(
                            osb_d[:, r, d0:d0 + ng, :], ops_v[:]
                        )
                        for i, d in enumerate(g):
                            if d in scalar_set:
                                nc.vector.tensor_add(
                                    osb_d[:, r, d, :], osb_d[:, r, d, :],
                                    brow_sb[:],
                                )
                nc.scalar.dma_start(o_t, osb[:, : R * N * D])
    nc.compile()
    return nc


def _prep_consts(weight, bias, indices):
    weight = np.asarray(weight, dtype=np.float32)
    bias = np.asarray(bias, dtype=np.float32)
    idx = [int(v) for v in np.asarray(indices).reshape(-1)]
    wdt = np.ascontiguousarray(weight.T)                      # [M, NREPS*N]
    brow = np.ascontiguousarray(
        np.broadcast_to(bias.reshape(1, N), (P, N))
    )
    ident = np.eye(P, dtype=np.float32)
    return wdt, brow, ident, idx


_NC_CACHE = {}


def kernel(f, weight, bias, indices, scalar_locs):
    f = np.asarray(f, dtype=np.float32)
    wdt, brow, ident, idx = _prep_consts(weight, bias, indices)
    scalar_set = set(int(v) for v in np.asarray(scalar_locs).reshape(-1))

    key = (f.shape[0], tuple(idx), tuple(sorted(scalar_set)))
    if key not in _NC_CACHE:
        b_core = f.shape[0] // N_CORES
        _NC_CACHE[key] = build_nc(b_core, idx, scalar_set)
    nc = _NC_CACHE[key]

    b_core = f.shape[0] // N_CORES
    in_maps = [
        {
            "f": f[i * b_core:(i + 1) * b_core],
            "wdt": wdt,
            "brow": brow,
            "ident": ident,
        }
        for i in range(N_CORES)
    ]
    res = run_bass_kernel_spmd(nc, in_maps, list(range(N_CORES)))
    return np.concatenate([r["out"] for r in res.results], axis=0)


# revision 28
# speedup vs baseline: 1.0253x; 1.0253x over previous
"""Trainium2 Bass kernel for nn_EquivariantLinear.

Reference computation (B=65536, IN_MULT=OUT_MULT=128, DIM=9, NREPS=3):
    w3 = weight.reshape(3, 128, 128)
    wd = w3[indices]                         # (9, 128, 128)
    out = einsum('dnm,bmd->bnd', wd, f)      # (B, 128, 9)
    out[..., scalar_locs] += bias            # bias on degree-0 column(s)

Strategy (data-parallel over batch, 8 NeuronCores):
  - Each core gets B/8 = 8192 batch rows; weight/bias replicated.
  - f is (b, m, d) with d innermost, so loading "m on partitions" directly
    would make 36-byte DMA runs.  Instead DMA contiguous 128-batch-row
    tiles [128b x 1152(m,d)], transpose each per-d [128b x 128m] slice on
    the TensorEngine (PSUM), copy back to SBUF, then matmul with the
    (transposed) per-irrep weight as the moving operand:
        O_d[b, n] = T_d.T @ wT_d,  T_d = F_d.T (stationary, [m, b])
    which lands the output with batch on partitions, so the store is a
    contiguous 128-row DMA as well.
  - Bias is fused into the PSUM->SBUF copy-out (tensor_add with a
    partition-replicated bias row) for the scalar (degree-0) columns.
"""

import numpy as np

import concourse.bass as bass
import concourse.tile as tile
from concourse import bacc, mybir
from concourse.bass_utils import run_bass_kernel_spmd

FP32 = mybir.dt.float32

N_CORES = 8
B_TOTAL = 65536
B_CORE = B_TOTAL // N_CORES
M = 128  # in_mult
N = 128  # out_mult
D = 9    # sum(2l+1)
NREPS = 3
P = 128  # partitions / batch tile


def tile_schedule(ntiles128):
    """(partitions, rows-per-partition) per supertile: 64-row half tiles at
    the very edges so the pipeline fills/drains fast, then R=4 descriptors
    (18.4 KB) in steady state."""
    if ntiles128 < 12 or ntiles128 % 4 != 0:
        return [(P, 1)] * ntiles128
    sched = [(64, 1), (64, 1), (P, 1), (P, 2)]      # 128+128+256 rows
    tail = [(P, 2), (P, 1), (64, 1), (64, 1)]
    rem = ntiles128 - 8                              # 4 units head + 4 tail
    return sched + [(P, 4)] * (rem // 4) + tail


def build_nc(b_core, idx, scalar_set):
    """Build the single-core Bass program (run SPMD on all cores)."""
    nc = bacc.Bacc(None, target_bir_lowering=False, debug=True)

    f = nc.dram_tensor("f", [b_core, M, D], FP32, kind="ExternalInput")
    wdt = nc.dram_tensor("wdt", [M, NREPS * N], FP32, kind="ExternalInput")
    brow = nc.dram_tensor("brow", [P, N], FP32, kind="ExternalInput")
    ident = nc.dram_tensor("ident", [P, P], FP32, kind="ExternalInput")
    out = nc.dram_tensor("out", [b_core, N, D], FP32, kind="ExternalOutput")

    sched = tile_schedule(b_core // P)
    groups = [list(range(0, 4)), list(range(4, 8)), [8]]

    with tile.TileContext(nc) as tc:
        with (
            tc.tile_pool(name="const", bufs=1) as cpool,
            tc.tile_pool(name="fin", bufs=4) as fpool,
            tc.tile_pool(name="tsb", bufs=6) as tspool,
            tc.tile_pool(name="osb", bufs=3) as ospool,
            tc.tile_pool(name="tps", bufs=4, space=bass.MemorySpace.PSUM) as tpsum,
            tc.tile_pool(name="ops", bufs=4, space=bass.MemorySpace.PSUM) as opsum,
        ):
            # constants go over the SWDGE (gpsimd) ring so the first f-tile
            # load is not queued behind them on the sync HWDGE ring; ident
            # first (the first transposes need it before anything else)
            id_sb = cpool.tile([P, P], FP32)
            nc.gpsimd.dma_start(id_sb[:], ident[:])
            wdt_sb = cpool.tile([M, NREPS * N], FP32)
            nc.gpsimd.dma_start(wdt_sb[:], wdt[:])
            brow_sb = cpool.tile([P, N], FP32)
            nc.gpsimd.dma_start(brow_sb[:], brow[:])

            cursor = 0
            for prow, R in sched:
                rows = prow * R
                f_t = f[cursor:cursor + rows].rearrange(
                    "(p r) m d -> p (r m d)", r=R
                )
                o_t = out[cursor:cursor + rows].rearrange(
                    "(p r) n d -> p (r n d)", r=R
                )
                cursor += rows
                fb = fpool.tile([P, 4 * M * D], FP32, tag="fb")
                nc.sync.dma_start(fb[:prow, : R * M * D], f_t)
                fb_d = fb[:prow, : R * M * D].rearrange(
                    "p (r m d) -> p r d m", d=D, r=R
                )
                osb = ospool.tile([P, 4 * N * D], FP32, tag="osb")
                osb_d = osb[:prow, : R * N * D].rearrange(
                    "p (r n d) -> p r d n", d=D, r=R
                )

                for r in range(R):
                    for g in groups:
                        ng = len(g)
                        d0 = g[0]
                        # per-d transposes of [prow x 128m] slices, packed
                        # tightly (prow cols each) into one bank
                        tps = tpsum.tile([P, 512], FP32, tag="tp")
                        for i, d in enumerate(g):
                            nc.tensor.transpose(
                                tps[:, i * prow:(i + 1) * prow],
                                fb_d[:, r, d, :],
                                id_sb[:prow, :prow],
                            )
                        tsb = tspool.tile([P, 512], FP32, tag="tsb")
                        nc.scalar.copy(tsb[:, : ng * prow], tps[:, : ng * prow])
                        # matmuls: O_d = T_d.T @ wT_{idx[d]} -> [b, n] in PSUM
                        ops = opsum.tile([P, 512], FP32, tag="op")
                        for i, d in enumerate(g):
                            nc.tensor.matmul(
                                ops[:prow, i * P:(i + 1) * P],
                                tsb[:, i * prow:(i + 1) * prow],
                                wdt_sb[:, idx[d] * N:(idx[d] + 1) * N],
                                start=True,
                                stop=True,
                            )
                        # copy out to SBUF, interleaving d back into (n d)
                        # order; runs of non-scalar d's in one copy, scalar
                        # d's fused with the bias add
                        ops_v = ops[:prow, : ng * P].rearrange(
                            "p (i n) -> p i n", i=ng
                        )
                        i = 0
                        while i < ng:
                            if g[i] in scalar_set:
                                nc.vector.tensor_add(
                                    osb_d[:, r, d0 + i, :], ops_v[:, i, :],
                                    brow_sb[:prow, :],
                                )
                                i += 1
                            else:
                                j = i
                                while j < ng and g[j] not in scalar_set:
                                    j += 1
                                nc.vector.tensor_copy(
                                    osb_d[:, r, d0 + i:d0 + j, :],
                                    ops_v[:, i:j, :],
                                )
                                i = j
                nc.scalar.dma_start(o_t, osb[:prow, : R * N * D])
    nc.compile()
    return nc


def _prep_consts(weight, bias, indices):
    weight = np.asarray(weight, dtype=np.float32)
    bias = np.asarray(bias, dtype=np.float32)
    idx = [int(v) for v in np.asarray(indices).reshape(-1)]
    wdt = np.ascontiguousarray(weight.T)                      # [M, NREPS*N]
    brow = np.ascontiguousarray(
        np.broadcast_to(bias.reshape(1, N), (P, N))
    )
    ident = np.eye(P, dtype=np.float32)
    return wdt, brow, ident, idx


_NC_CACHE = {}


def kernel(f, weight, bias, indices, scalar_locs):
    f = np.asarray(f, dtype=np.float32)
    wdt, brow, ident, idx = _prep_consts(weight, bias, indices)
    scalar_set = set(int(v) for v in np.asarray(scalar_locs).reshape(-1))

    key = (f.shape[0], tuple(idx), tuple(sorted(scalar_set)))
    if key not in _NC_CACHE:
        b_core = f.shape[0] // N_CORES
        _NC_CACHE[key] = build_nc(b_core, idx, scalar_set)
    nc = _NC_CACHE[key]

    b_core = f.shape[0] // N_CORES
    in_maps = [
        {
            "f": f[i * b_core:(i + 1) * b_core],
            "wdt": wdt,
            "brow": brow,
            "ident": ident,
        }
        for i in range(N_CORES)
    ]
    res = run_bass_kernel_spmd(nc, in_maps, list(range(N_CORES)))
    return np.concatenate([r["out"] for r in res.results], axis=0)
